# revision 1
# baseline (speedup 1.0000x reference)
"""Trainium2 Bass kernel for nn_GAT_with_LSTM (2-layer LSTM -> 8-head GAT -> GAT out).

Sharding: node/row dimension split across 8 cores (512 rows each). Each core:
  - runs the LSTM for its own 512 nodes (gates [48, n] layout, nodes on free dim),
  - AllGathers the LSTM features hT [96, 512] -> hT_full [96, 4096],
  - computes per-head Wh/f1/f2 (replicated small matmuls),
  - computes its row-block of the masked-softmax attention in transposed
    layout eT[j, i] = exp(leakyrelu(f1_i + f2_j)) * mask[i, j], accumulating
    att@[Wh|1] (numerator + denominator together) through the PE,
  - AllGathers the per-block output-layer Wh_out, runs the output GAT layer,
    and writes log_softmax(elu(out)) for its own rows.

Softmax max-subtraction is skipped: attention logits here are O(1) (weights
are ~0.1-scale Xavier inits), so exp() cannot overflow, and softmax is
shift-invariant so results match the reference to fp32 rounding.
"""

import json

import numpy as np

import bass_rust
import concourse.bass as bass
import concourse.tile as tile
from concourse import mybir
from concourse.bass_utils import run_bass_kernel_spmd
from concourse.masks import make_identity

F32 = mybir.dt.float32
F32R = mybir.dt.float32r
BF16 = mybir.dt.bfloat16
I32 = mybir.dt.int32
AF = mybir.ActivationFunctionType
OP = mybir.AluOpType

NCORES = 8
N = 4096
R = N // NCORES          # 512 rows per core
SEQ, NIN, LH = 8, 2, 12
G4 = 4 * LH              # 48 gate rows
FEAT = SEQ * LH          # 96
NHID, NHEADS, NCLASS = 64, 8, 16
ALPHA = 0.2
NJC = N // 128           # 32 j-chunks
NSUB = R // 128          # 4 row sub-blocks per core


def _split_sync_waits(nc, max_waits=1):
    """This walrus build rejects >1 sync wait per TPB_CTRL instruction
    ("Too many sync wait commands"). Move excess waits onto NoOps inserted
    just before; same-engine program order preserves the semantics."""
    m = json.loads(bass_rust.module_to_json_string(nc.m))
    ctr = 0
    for fn in m["functions"]:
        for bb in fn["blocks"]:
            out = []
            for inst in bb["instructions"]:
                si = inst.get("sync_info")
                ow = (si or {}).get("on_wait") or []
                if len(ow) > max_waits:
                    excess, keep = ow[:-max_waits], ow[-max_waits:]
                    for i in range(0, len(excess), max_waits):
                        ctr += 1
                        out.append({
                            "engine": inst["engine"], "ins": [], "outs": [],
                            "name": f"wsplit-{ctr}", "opcode": "NoOp",
                            "sync_info": {"on_update": [],
                                          "on_wait": excess[i:i + max_waits]},
                        })
                    si["on_wait"] = keep
                out.append(inst)
            bb["instructions"] = out
    nc.m = bass_rust.module_from_json_bytes(json.dumps(m).encode())


def _lstm_layer(nc, lay, p1, hpool, lwork, psg, xin_slices, wihT, whhT, b,
                h_copy_to=None, h_step_hook=None):
    """One LSTM layer over SEQ steps. xin_slices(t) -> rhs AP [in, R].
    The padded gate layout puts i/f/g/o at partition rows 0/32/64/96 (compute
    engines require 32-aligned partition bases; weights are host-padded to
    match). Returns the list of h tiles (base-partition 0, rotating slots).
    h_copy_to(t), if given, receives a DMA copy of each step's h."""
    c_t = p1.tile([LH, R], F32, tag=f"c{lay}", name=f"c{lay}")
    hs = []
    hprev = None
    for t in range(SEQ):
        g = psg.tile([128, R], F32, tag="g", name=f"g{lay}_{t}")
        nc.tensor.matmul(g, wihT, xin_slices(t), start=True, stop=(t == 0))
        if t > 0:
            nc.tensor.matmul(g, whhT, hprev, start=False, stop=True)
        # f-gate first: it heads the c-recurrence critical path
        sig_f = lwork.tile([LH, R], F32, tag="sig_f", name=f"sf{lay}_{t}")
        sig_i = lwork.tile([LH, R], F32, tag="sig_i", name=f"si{lay}_{t}")
        tan_g = lwork.tile([LH, R], F32, tag="tan_g", name=f"tg{lay}_{t}")
        sig_o = lwork.tile([LH, R], F32, tag="sig_o", name=f"so{lay}_{t}")
        nc.scalar.activation(sig_f, g[32:32 + LH, :], AF.Sigmoid,
                             bias=b[32:32 + LH, :])
        nc.scalar.activation(sig_i, g[0:LH, :], AF.Sigmoid, bias=b[0:LH, :])
        nc.scalar.activation(tan_g, g[64:64 + LH, :], AF.Tanh,
                             bias=b[64:64 + LH, :])
        nc.scalar.activation(sig_o, g[96:96 + LH, :], AF.Sigmoid,
                             bias=b[96:96 + LH, :])
        ig = lwork.tile([LH, R], F32, tag="ig", name=f"ig{lay}_{t}")
        nc.vector.tensor_mul(ig, sig_i, tan_g)
        if t == 0:
            nc.vector.tensor_copy(c_t, ig)
        else:
            nc.vector.tensor_mul(c_t, sig_f, c_t)
            nc.vector.tensor_add(c_t, c_t, ig)
        th = lwork.tile([LH, R], F32, tag="th", name=f"th{lay}_{t}")
        nc.scalar.activation(th, c_t, AF.Tanh)
        h = hpool.tile([LH, R], F32, tag=f"h{lay}", name=f"h{lay}_{t}")
        nc.vector.tensor_mul(h, sig_o, th)
        if h_copy_to is not None:
            nc.sync.dma_start(out=h_copy_to(t), in_=h)
        if h_step_hook is not None:
            h_step_hook(t, h)
        hs.append(h)
        hprev = h
    return hs


GRP = 8  # j-chunks per wide ACT op


def _attention(nc, awork, pspv, f1b, f2cols, maskT, wpv, ncols, pfx):
    """Masked-softmax attention for this core's 512-row block. Returns the
    PSUM tile [128, NSUB, ncols+1]; col ncols is the softmax denominator.

    z = f1 + f2 is pre-added per chunk on DVE/GpSimd (alternating) so the
    Prelu/Exp ACT passes run bias-free over GRP-chunk-wide tiles, amortizing
    the per-op ACT overhead."""
    pv = pspv.tile([128, NSUB, ncols + 1], F32, tag="pv", name=f"pv_{pfx}")
    for cg in range(NJC // GRP):
        zq = awork.tile([128, GRP, R], F32, tag="zq", name=f"zq_{pfx}_{cg}")
        for q in range(GRP):
            c = cg * GRP + q
            eng = nc.vector if c % 2 == 0 else nc.gpsimd
            eng.tensor_scalar(zq[:, q, :], f1b, scalar1=f2cols[:, c, :],
                              scalar2=None, op0=OP.add)
        nc.scalar.activation(zq, zq, AF.Prelu, alpha=ALPHA)
        e2 = awork.tile([128, GRP, R], BF16, tag="e2", name=f"e2_{pfx}_{cg}")
        nc.scalar.activation(e2, zq, AF.Exp)
        e3 = awork.tile([128, GRP, R], BF16, tag="e3", name=f"e3_{pfx}_{cg}")
        nc.vector.tensor_mul(e3, e2, maskT[:, cg * GRP:(cg + 1) * GRP, :])
        for q in range(GRP):
            c = cg * GRP + q
            for s in range(NSUB):
                nc.tensor.matmul(pv[:, s, :], e3[:, q, 128 * s:128 * (s + 1)],
                                 wpv[:, c, :], start=(c == 0),
                                 stop=(c == NJC - 1))
    return pv


def _elu_into(nc, awork, dst, z, pfx):
    """dst = elu(z) = min(exp(z),1)-1 + max(z,0), elementwise."""
    ez = awork.tile(list(z.shape), F32, tag="elu_ez", name=f"ez_{pfx}")
    nc.scalar.activation(ez, z, AF.Exp)
    nc.vector.tensor_scalar(ez, ez, scalar1=1.0, scalar2=-1.0,
                            op0=OP.min, op1=OP.add)
    zr = awork.tile(list(z.shape), F32, tag="elu_zr", name=f"zr_{pfx}")
    nc.vector.tensor_scalar(zr, z, scalar1=0.0, scalar2=None, op0=OP.max)
    nc.vector.tensor_add(dst, ez, zr)


def _build_program():
    nc = bass.Bass()

    xT = nc.dram_tensor("xT", [NIN, SEQ, R], F32, kind="ExternalInput")
    adjb = nc.dram_tensor("adjb", [R, N], I32, kind="ExternalInput")
    wih0T = nc.dram_tensor("wih0T", [NIN, 128], F32, kind="ExternalInput")
    whh0T = nc.dram_tensor("whh0T", [LH, 128], F32, kind="ExternalInput")
    wih1T = nc.dram_tensor("wih1T", [LH, 128], F32, kind="ExternalInput")
    whh1T = nc.dram_tensor("whh1T", [LH, 128], F32, kind="ExternalInput")
    b0d = nc.dram_tensor("b0", [128, 1], F32, kind="ExternalInput")
    b1d = nc.dram_tensor("b1", [128, 1], F32, kind="ExternalInput")
    wcat = nc.dram_tensor("wcat", [NHEADS, FEAT, NHID + 2], F32, kind="ExternalInput")
    wocat = nc.dram_tensor("wocat", [NHEADS * NHID, NCLASS + 2], F32, kind="ExternalInput")
    outb = nc.dram_tensor("outb", [R, NCLASS], F32, kind="ExternalOutput")

    with tile.TileContext(nc) as tc:
        with tc.tile_pool(name="cst", bufs=1) as cst, \
             tc.tile_pool(name="psg", bufs=2, space="PSUM") as psg, \
             tc.tile_pool(name="pstr", bufs=1, space="PSUM") as pstr, \
             tc.tile_pool(name="pswh", bufs=2, space="PSUM") as pswh, \
             tc.tile_pool(name="psf1", bufs=1, space="PSUM") as psf1, \
             tc.tile_pool(name="pspv", bufs=2, space="PSUM") as pspv, \
             tc.tile_pool(name="dram", bufs=1, space="DRAM") as dram:

            ident = cst.tile([128, 128], F32)
            make_identity(nc, ident)
            ones1 = cst.tile([1, 128], F32)
            nc.vector.memset(ones1, 1.0)
            maskT = cst.tile([128, NJC, R], BF16)
            hT_own = cst.tile([FEAT, R], F32)
            hT_full = cst.tile([FEAT, N], F32)

            g1in = dram.tile([FEAT, R], BF16)
            g1out = dram.tile([NCORES * FEAT, R], BF16, addr_space="Shared")
            g2in = dram.tile([R, NCLASS + 2], F32)
            g2out = dram.tile([N, NCLASS + 2], F32, addr_space="Shared")

            # ======== Phase 1: LSTM (own nodes) + mask build + gather =======
            with tc.tile_pool(name="p1", bufs=1) as p1, \
                 tc.tile_pool(name="hpool0", bufs=SEQ) as hpool0, \
                 tc.tile_pool(name="hpool1", bufs=3) as hpool1, \
                 tc.tile_pool(name="lwork", bufs=4) as lwork, \
                 tc.tile_pool(name="mstage", bufs=1) as mstage:

                xT_sb = p1.tile([NIN, SEQ, R], F32)
                nc.sync.dma_start(out=xT_sb, in_=xT[:])
                w0 = p1.tile([NIN, 128], F32)
                w0h = p1.tile([LH, 128], F32)
                w1 = p1.tile([LH, 128], F32)
                w1h = p1.tile([LH, 128], F32)
                b0 = p1.tile([128, 1], F32)
                b1 = p1.tile([128, 1], F32)
                for dst, src in ((w0, wih0T), (w0h, whh0T), (w1, wih1T),
                                 (w1h, whh1T), (b0, b0d), (b1, b1d)):
                    nc.sync.dma_start(out=dst, in_=src[:])

                h0s = _lstm_layer(nc, 0, p1, hpool0, lwork, psg,
                                  lambda t: xT_sb[:, t, :], w0, w0h, b0)
                def _h1_hook(t, h):
                    hb = lwork.tile([LH, R], BF16, tag="h1b", name=f"h1b{t}")
                    nc.vector.tensor_copy(hb, h)
                    nc.sync.dma_start(out=g1in[LH * t:LH * (t + 1), :], in_=hb)
                    if t == SEQ - 1:
                        nc.gpsimd.collective_compute(
                            "AllGather", OP.bypass,
                            replica_groups=[list(range(NCORES))],
                            ins=[g1in[:].opt()], outs=[g1out[:].opt()])

                _lstm_layer(nc, 1, p1, hpool1, lwork, psg,
                            lambda t: h0s[t], w1, w1h, b1,
                            h_copy_to=lambda t: hT_own[LH * t:LH * (t + 1), :],
                            h_step_hook=_h1_hook)

                # mask build: cast own adj rows to bf16, bounce via DRAM,
                # transpose with the DMA xbar (no PE/ACT involvement)
                adjbf = dram.tile([R, N], BF16)
                for rc in range(NSUB):
                    ai = mstage.tile([128, N], I32, tag="ai", name=f"ai{rc}")
                    nc.gpsimd.dma_start(out=ai, in_=adjb[128 * rc:128 * (rc + 1), :])
                    af = mstage.tile([128, N], BF16, tag="af", name=f"af{rc}")
                    nc.vector.tensor_copy(af, ai)
                    nc.sync.dma_start(out=adjbf[128 * rc:128 * (rc + 1), :],
                                      in_=af)
                    nc.sync.dma_start_transpose(
                        maskT[:, :, 128 * rc:128 * (rc + 1)],
                        adjbf[128 * rc:128 * (rc + 1), :])

                hT_fb = p1.tile([FEAT, N], BF16)
                for bb in range(NCORES):
                    nc.sync.dma_start(out=hT_fb[:, R * bb:R * (bb + 1)],
                                      in_=g1out[FEAT * bb:FEAT * (bb + 1), :])
                nc.vector.tensor_copy(hT_full, hT_fb)

            # ======== Phase 2: 8 GAT heads + output GAT layer ===============
            with tc.tile_pool(name="att", bufs=1) as att, \
                 tc.tile_pool(name="hw", bufs=2) as hw, \
                 tc.tile_pool(name="awork", bufs=2) as awork:

                hcat = att.tile([128, NSUB, NHEADS * NHID], F32)

                for h in range(NHEADS):
                    whpv = hw.tile([128, NJC, NHID + 1], BF16, tag="whpv",
                                   name=f"whpv{h}")
                    nc.vector.memset(whpv[:, :, NHID:NHID + 1], 1.0)
                    f2cols = hw.tile([128, NJC, 1], F32, tag="f2cols",
                                     name=f"f2cols{h}")
                    f1b_sb = hw.tile([128, R], F32, tag="f1b", name=f"f1b{h}")
                    wc = awork.tile([FEAT, NHID + 2], F32, tag="wc",
                                    name=f"wc{h}")
                    nc.sync.dma_start(out=wc, in_=wcat[h])
                    # f1 (own rows) -> broadcast across partitions
                    pf1 = psf1.tile([1, R], F32, tag="f1r", name=f"pf1_{h}")
                    nc.tensor.matmul(pf1, wc[0:64, NHID:NHID + 1],
                                     hT_own[0:64, :], start=True, stop=False)
                    nc.tensor.matmul(pf1, wc[64:FEAT, NHID:NHID + 1],
                                     hT_own[64:FEAT, :], start=False, stop=True)
                    f1row = awork.tile([1, R], F32, tag="f1row", name=f"f1row{h}")
                    nc.scalar.copy(f1row, pf1)
                    pf1b = psf1.tile([128, R], F32, tag="f1r", name=f"pf1b_{h}")
                    nc.tensor.matmul(pf1b, ones1, f1row, start=True, stop=True)
                    nc.scalar.copy(f1b_sb, pf1b)
                    # Wh (+f2) for all nodes, replicated
                    for c in range(NJC):
                        pw = pswh.tile([128, NHID + 2], F32, tag="wh",
                                       name=f"pw{h}_{c}")
                        nc.tensor.matmul(pw, hT_full[0:64, 128 * c:128 * (c + 1)],
                                         wc[0:64, :], start=True, stop=False)
                        nc.tensor.matmul(pw, hT_full[64:FEAT, 128 * c:128 * (c + 1)],
                                         wc[64:FEAT, :], start=False, stop=True)
                        nc.vector.tensor_copy(whpv[:, c, 0:NHID], pw[:, 0:NHID])
                        nc.vector.tensor_copy(f2cols[:, c, :], pw[:, NHID + 1:NHID + 2])

                    pv = _attention(nc, awork, pspv, f1b_sb, f2cols, maskT,
                                    whpv, NHID, f"h{h}")
                    zall = awork.tile([128, NSUB, NHID], F32, tag="zall",
                                      name=f"zall{h}")
                    for s in range(NSUB):
                        rcp = awork.tile([128, 1], F32, tag="rcp",
                                         name=f"rcp{h}_{s}")
                        nc.vector.reciprocal(rcp, pv[:, s, NHID:NHID + 1])
                        nc.vector.tensor_scalar_mul(zall[:, s, :],
                                                    pv[:, s, 0:NHID], rcp)
                    _elu_into(nc, awork, hcat[:, :, NHID * h:NHID * (h + 1)],
                              zall, f"h{h}")

                # ---- output layer ----
                hcatT = att.tile([128, NSUB, R], F32)
                for s in range(NSUB):
                    for fc in range(NSUB):
                        ptr = pstr.tile([128, 128], F32, tag="tr",
                                        name=f"trh{s}_{fc}")
                        nc.tensor.transpose(
                            ptr, hcat[:, s, 128 * fc:128 * (fc + 1)], ident)
                        nc.scalar.copy(hcatT[:, fc, 128 * s:128 * (s + 1)], ptr)

                woc = att.tile([128, NSUB, NCLASS + 2], F32)
                nc.sync.dma_start(
                    out=woc, in_=wocat.rearrange("(c p) f -> p c f", p=128))

                g2stage = awork.tile([128, NSUB, NCLASS + 2], F32, tag="g2stage")
                for s in range(NSUB):
                    pwo = pswh.tile([128, NCLASS + 2], F32, tag="wh",
                                    name=f"pwo{s}")
                    for fc in range(NSUB):
                        nc.tensor.matmul(pwo, hcatT[:, fc, 128 * s:128 * (s + 1)],
                                         woc[:, fc, :], start=(fc == 0),
                                         stop=(fc == NSUB - 1))
                    nc.scalar.copy(g2stage[:, s, :], pwo)
                nc.sync.dma_start(
                    out=g2in[:].rearrange("(c p) f -> p c f", p=128),
                    in_=g2stage)

                pf1o = psf1.tile([1, R], F32, tag="f1r", name="pf1o")
                for fc in range(NSUB):
                    nc.tensor.matmul(pf1o, woc[:, fc, NCLASS:NCLASS + 1],
                                     hcatT[:, fc, :], start=(fc == 0),
                                     stop=(fc == NSUB - 1))
                f1orow = awork.tile([1, R], F32, tag="f1row", name="f1orow")
                nc.scalar.copy(f1orow, pf1o)
                pf1ob = psf1.tile([128, R], F32, tag="f1r", name="pf1ob")
                nc.tensor.matmul(pf1ob, ones1, f1orow, start=True, stop=True)
                f1ob = hw.tile([128, R], F32, tag="f1b", name="f1ob")
                nc.scalar.copy(f1ob, pf1ob)

                nc.gpsimd.collective_compute(
                    "AllGather", OP.bypass,
                    replica_groups=[list(range(NCORES))],
                    ins=[g2in[:].opt()], outs=[g2out[:].opt()])

                wopv = hw.tile([128, NJC, NCLASS + 1], BF16, tag="whpv",
                               name="wopv")
                nc.vector.memset(wopv[:, :, NCLASS:NCLASS + 1], 1.0)
                f2ocols = hw.tile([128, NJC, 1], F32, tag="f2cols",
                                  name="f2ocols")
                g2r = g2out[:].rearrange("(c p) f -> p c f", p=128)
                wof = awork.tile([128, NJC, NCLASS], F32, tag="wof")
                nc.sync.dma_start(out=wof, in_=g2r[:, :, 0:NCLASS])
                nc.vector.tensor_copy(wopv[:, :, 0:NCLASS], wof)
                nc.sync.dma_start(out=f2ocols,
                                  in_=g2r[:, :, NCLASS + 1:NCLASS + 2])

                pvo = _attention(nc, awork, pspv, f1ob, f2ocols, maskT, wopv,
                                 NCLASS, "o")
                zoall = awork.tile([128, NSUB, NCLASS], F32, tag="zoall")
                for s in range(NSUB):
                    rcp = awork.tile([128, 1], F32, tag="rcp", name=f"rcpo{s}")
                    nc.vector.reciprocal(rcp, pvo[:, s, NCLASS:NCLASS + 1])
                    nc.vector.tensor_scalar_mul(zoall[:, s, :],
                                                pvo[:, s, 0:NCLASS], rcp)
                ziall = awork.tile([128, NSUB, NCLASS], F32, tag="ziall")
                _elu_into(nc, awork, ziall, zoall, "oall")
                for s in range(NSUB):
                    zi = ziall[:, s, :]
                    edump = awork.tile([128, NCLASS], F32, tag="edump",
                                       name=f"ed{s}")
                    ssum = awork.tile([128, 1], F32, tag="ssum", name=f"ss{s}")
                    nc.scalar.activation(edump, zi, AF.Exp, accum_out=ssum)
                    lns = awork.tile([128, 1], F32, tag="lns", name=f"ln{s}")
                    nc.scalar.activation(lns, ssum, AF.Ln)
                    ls = awork.tile([128, NCLASS], F32, tag="ls", name=f"ls{s}")
                    nc.vector.tensor_scalar(ls, zi, scalar1=lns, scalar2=None,
                                            op0=OP.subtract)
                    nc.sync.dma_start(out=outb[128 * s:128 * (s + 1), :],
                                      in_=ls)

    _split_sync_waits(nc)
    return nc


_NC_CACHE = None


def kernel(x, adj, Wih0, Whh0, bih0, bhh0, Wih1, Whh1, bih1, bhh1,
           W_heads, a_heads, W_out, a_out):
    global _NC_CACHE
    if _NC_CACHE is None:
        _NC_CACHE = _build_program()
    nc = _NC_CACHE

    x = np.asarray(x, np.float32)
    adj = np.ascontiguousarray(np.asarray(adj, np.int32))
    W_heads = np.asarray(W_heads, np.float32)
    a_heads = np.asarray(a_heads, np.float32)
    W_out = np.asarray(W_out, np.float32)
    a_out = np.asarray(a_out, np.float32)

    wcat = np.concatenate(
        [W_heads,
         W_heads @ a_heads[:, :NHID, :],
         W_heads @ a_heads[:, NHID:, :]], axis=2).astype(np.float32)
    wocat = np.concatenate(
        [W_out, W_out @ a_out[:NCLASS], W_out @ a_out[NCLASS:]],
        axis=1).astype(np.float32)
    def pad_gates_T(w):
        # [4H, in] -> transposed+padded [in, 128]: gate k rows at 32k..32k+11
        w = np.asarray(w, np.float32)
        out = np.zeros((w.shape[1], 128), np.float32)
        for k in range(4):
            out[:, 32 * k:32 * k + LH] = w[LH * k:LH * (k + 1), :].T
        return out

    def pad_bias(ba, bb):
        b = np.asarray(ba, np.float32) + np.asarray(bb, np.float32)
        out = np.zeros((128, 1), np.float32)
        for k in range(4):
            out[32 * k:32 * k + LH, 0] = b[LH * k:LH * (k + 1)]
        return out

    common = {
        "wih0T": pad_gates_T(Wih0),
        "whh0T": pad_gates_T(Whh0),
        "wih1T": pad_gates_T(Wih1),
        "whh1T": pad_gates_T(Whh1),
        "b0": pad_bias(bih0, bhh0),
        "b1": pad_bias(bih1, bhh1),
        "wcat": np.ascontiguousarray(wcat),
        "wocat": np.ascontiguousarray(wocat),
    }
    in_maps = []
    for i in range(NCORES):
        blk = slice(R * i, R * (i + 1))
        in_maps.append({
            "xT": np.ascontiguousarray(x[blk].transpose(2, 1, 0)),
            "adjb": np.ascontiguousarray(adj[blk]),
            **common,
        })

    res = run_bass_kernel_spmd(nc, in_maps, list(range(NCORES)), **_RUN_KWARGS)
    global _LAST_RESULTS
    _LAST_RESULTS = res
    return np.concatenate([res.results[i]["outb"] for i in range(NCORES)], axis=0)


_RUN_KWARGS = {}
_LAST_RESULTS = None



# revision 28
# speedup vs baseline: 2.3618x; 2.3618x over previous
"""Trainium2 Bass kernel for nn_GAT_with_LSTM (2-layer LSTM -> 8-head GAT -> GAT out).

Sharding: node/row dimension split across 8 cores (512 rows each).

Key algebraic restructuring of the GAT attention (vs. direct
exp(leakyrelu(f1+f2)) evaluation): with z = f1_i + f2_j and slope a,
    leakyrelu(z) = max(z, a*z)  =>  e = exp(lrelu(z)) = max(exp(z), exp(a*z)).
Softmax rows are invariant to any per-row (i) factor, so divide by
exp(a*f1_i):
    e'_ij = max(u_i * V_j, D_j),   u = exp((1-a)*f1), V = exp(f2), D = exp(a*f2).
This removes every full-matrix transcendental: exp() runs only on the rank-1
factors. Per 128-column chunk the e-row-block is built one of three ways,
chosen to balance engines:
  - DVE:  t = (ub * V_j) max D_j      (dual-op tensor_scalar, bf16 4x mode)
  - Pool: same op at 1x
  - ACT:  r = relu(V_j * ub - D_j)    (per-partition scale/bias APs); the
          missing mask*D_j term is added back on the PE as
          maskT_chunk @ (D (*) wpv), exact since mask is 0/1:
          mask*max(uV,D) = mask*r + mask*D.
Then one wide bf16 tensor_tensor multiplies the mask in, and the PE
accumulates numerator and denominator together (wpv's last column is ones).

The mask arrives host-side pre-transposed and pre-cast to bf16 (adj[blk].T).
The LSTM packs two 256-node groups into the partition dim with
block-diagonal host-packed weights (one sigmoid op covers i+f, another o),
runs bf16 matmuls, and software-pipelines layer 1 one step behind layer 0.
The LSTM feature AllGather ships fp8e4 (f1/f2 logit noise ~0.05 and Wh value
noise average out across the ~2048-wide attention sums).

Softmax max-subtraction is skipped: attention logits are O(1) (0.1-scale
Xavier weights), exp cannot overflow, softmax is shift-invariant.
"""

import json

import numpy as np
import ml_dtypes

import bass_rust
import concourse.bass as bass
import concourse.tile as tile
from concourse import mybir
from concourse.bass_utils import run_bass_kernel_spmd
from concourse.masks import make_identity

F32 = mybir.dt.float32
BF16 = mybir.dt.bfloat16
FP8 = mybir.dt.float8e4
I32 = mybir.dt.int32
AF = mybir.ActivationFunctionType
OP = mybir.AluOpType

NCORES = 8
N = 4096
R = N // NCORES          # 512 rows per core
SEQ, NIN, LH = 8, 2, 12
FEAT = SEQ * LH          # 96
NHID, NHEADS, NCLASS = 64, 8, 16
ALPHA = 0.2
NJC = N // 128           # 32 j-chunks
NSUB = R // 128          # 4 row sub-blocks per core
GRP = 8                  # j-chunks per wide mask-mul
RH = R // 2              # 256-node half (LSTM partition packing)

# P1 engine assignment within each GRP of 8 chunks
Q_DVE = (0, 1)
Q_ACT = (4, 5)
# remaining q go to Pool


def _split_sync_waits(nc, max_waits=1):
    """This walrus build rejects >1 sync wait per TPB_CTRL instruction
    ("Too many sync wait commands"). Move excess waits onto NoOps inserted
    just before; same-engine program order preserves the semantics."""
    m = json.loads(bass_rust.module_to_json_string(nc.m))
    ctr = 0
    for fn in m["functions"]:
        for bb in fn["blocks"]:
            out = []
            for inst in bb["instructions"]:
                si = inst.get("sync_info")
                ow = (si or {}).get("on_wait") or []
                if len(ow) > max_waits:
                    excess, keep = ow[:-max_waits], ow[-max_waits:]
                    for i in range(0, len(excess), max_waits):
                        ctr += 1
                        out.append({
                            "engine": inst["engine"], "ins": [], "outs": [],
                            "name": f"wsplit-{ctr}", "opcode": "NoOp",
                            "sync_info": {"on_update": [],
                                          "on_wait": excess[i:i + max_waits]},
                        })
                    si["on_wait"] = keep
                out.append(inst)
            bb["instructions"] = out
    nc.m = bass_rust.module_from_json_bytes(json.dumps(m).encode())


RQ = RH // 2  # 128-node quarter: free-dim half of a packed 256 pair


class _LstmState:
    def __init__(self, lay, p1, wih, whh, b, xin):
        self.lay, self.wih, self.whh, self.b, self.xin = lay, wih, whh, b, xin
        # c lives at partition base 32 so TensorTensor partners the f-gate
        # slice (walrus requires equal SBUF base partitions for both inputs)
        self.c01 = [p1.tile([56, RQ], F32, tag=f"c{lay}_{hf}",
                            name=f"c{lay}_{hf}")[32:56, :] for hf in (0, 1)]
        self.hprev = [None, None]


def _lstm_step(nc, st, t, hf, psg, lwork, hpool):
    """One packed LSTM step for free-half hf. Partition layout (two 256-node
    groups packed): i at rows 0:24, f at 32:56, o at 64:88, g at 96:120.
    One sigmoid covers i/f/o. The two free-halves are independent chains, so
    four chains (2 layers x 2 halves) pipeline across the engines."""
    lay = st.lay
    g = psg.tile([128, RQ], F32, tag=f"g{hf}", name=f"g{lay}_{t}_{hf}")
    nc.tensor.matmul(g, st.wih, st.xin(t, hf), start=True, stop=(t == 0))
    if t > 0:
        nc.tensor.matmul(g, st.whh, st.hprev[hf], start=False, stop=True)
    sfi = lwork.tile([88, RQ], F32, tag=f"sfi{hf}", name=f"sfi{lay}_{t}_{hf}")
    nc.scalar.activation(sfi, g[0:88, :], AF.Sigmoid, bias=st.b[0:88, :])
    tg = lwork.tile([24, RQ], F32, tag=f"tg{hf}", name=f"tg{lay}_{t}_{hf}")
    nc.scalar.activation(tg, g[96:120, :], AF.Tanh, bias=st.b[96:120, :])
    c01 = st.c01[hf]
    if t == 0:
        nc.gpsimd.tensor_mul(c01, sfi[0:24, :], tg)
    else:
        # ig at base 32 to partner c01; th at base 64 to partner the o slice
        ig = lwork.tile([56, RQ], F32, tag=f"ig{hf}",
                        name=f"ig{lay}_{t}_{hf}")[32:56, :]
        nc.gpsimd.tensor_mul(ig, sfi[0:24, :], tg)
        nc.gpsimd.tensor_mul(c01, sfi[32:56, :], c01)
        nc.gpsimd.tensor_add(c01, c01, ig)
    th = lwork.tile([88, RQ], F32, tag=f"th{hf}",
                    name=f"th{lay}_{t}_{hf}")[64:88, :]
    nc.scalar.activation(th, c01, AF.Tanh)
    h = hpool.tile([24, RQ], BF16, tag=f"h{lay}_{hf}", name=f"h{lay}_{t}_{hf}")
    nc.vector.tensor_mul(h, sfi[64:88, :], th)
    st.hprev[hf] = h
    return h


def _attention(nc, awork, pspv, ub, vcols, dcols, negd, wpv, wpv_d, maskT,
               ncols, pfx):
    """Masked-softmax attention numerator+denominator for this core's
    512-row block. Returns PSUM tile [128, NSUB, ncols+1]; col ncols is the
    softmax denominator (wpv's last column is memset ones)."""
    pv = pspv.tile([128, NSUB, ncols + 1], F32, tag="pv", name=f"pv_{pfx}")
    for cg in range(NJC // GRP):
        tq = awork.tile([128, GRP, R], BF16, tag="tq", name=f"tq_{pfx}_{cg}")
        for q in range(GRP):
            c = cg * GRP + q
            if q in Q_ACT:
                nc.scalar.activation(tq[:, q, :], ub, AF.Relu,
                                     scale=vcols[:, c, :],
                                     bias=negd[:, c, :])
            else:
                eng = nc.vector if q in Q_DVE else nc.gpsimd
                eng.tensor_scalar(tq[:, q, :], ub, scalar1=vcols[:, c, :],
                                  scalar2=dcols[:, c, :],
                                  op0=OP.mult, op1=OP.max)
        e3 = awork.tile([128, GRP, R], BF16, tag="e3", name=f"e3_{pfx}_{cg}")
        nc.vector.tensor_mul(e3, tq, maskT[:, cg * GRP:(cg + 1) * GRP, :])
        last = cg == NJC // GRP - 1
        for q in range(GRP):
            c = cg * GRP + q
            for s in range(NSUB):
                nc.tensor.matmul(pv[:, s, :], e3[:, q, 128 * s:128 * (s + 1)],
                                 wpv[:, c, :], start=(c == 0),
                                 stop=(last and q == GRP - 1))
            if q in Q_ACT:
                # mask*D_j correction for the relu form
                for s in range(NSUB):
                    nc.tensor.matmul(pv[:, s, :],
                                     maskT[:, c, 128 * s:128 * (s + 1)],
                                     wpv_d[:, cg * 2 + (q - Q_ACT[0]), :],
                                     start=False, stop=False)
    return pv


def _elu_into(nc, awork, dst, z, pfx):
    """dst = elu(z) = min(exp(z),1)-1 + max(z,0), elementwise."""
    ez = awork.tile(list(z.shape), F32, tag="elu_ez", name=f"ez_{pfx}")
    nc.scalar.activation(ez, z, AF.Exp)
    nc.gpsimd.tensor_scalar(ez, ez, scalar1=1.0, scalar2=-1.0,
                            op0=OP.min, op1=OP.add)
    zr = awork.tile(list(z.shape), F32, tag="elu_zr", name=f"zr_{pfx}")
    nc.gpsimd.tensor_scalar(zr, z, scalar1=0.0, scalar2=None, op0=OP.max)
    nc.vector.tensor_add(dst, ez, zr)


def _ubcast(nc, psf1, awork, ubpool, ones1, coefT, feats, nk, scale, pfx):
    """u = exp(scale * (coefT.T @ feats)) broadcast over partitions."""
    pf1 = psf1.tile([1, R], F32, tag="f1r", name=f"pf1_{pfx}")
    if nk == 1:
        nc.tensor.matmul(pf1, coefT, feats, start=True, stop=True)
    else:
        for fc in range(nk):
            nc.tensor.matmul(pf1, coefT[:, fc, :], feats[:, fc, :],
                             start=(fc == 0), stop=(fc == nk - 1))
    f1row = awork.tile([1, R], BF16, tag="f1row", name=f"f1row_{pfx}")
    nc.scalar.copy(f1row, pf1)
    pf1b = psf1.tile([128, R], F32, tag="f1r", name=f"pf1b_{pfx}")
    nc.tensor.matmul(pf1b, ones1, f1row, start=True, stop=True)
    ub = ubpool.tile([128, R], BF16, tag=f"ub_{pfx}", name=f"ub_{pfx}")
    nc.scalar.activation(ub, pf1b, AF.Exp, scale=scale)
    return ub


def _head_factors(nc, hw, f2cols, pfx):
    """V = exp(f2), D = exp(a*f2), negD = -D per chunk column."""
    vcols = hw.tile([128, NJC, 1], F32, tag="vcols", name=f"vcols{pfx}")
    nc.scalar.activation(vcols, f2cols, AF.Exp)
    dcols = hw.tile([128, NJC, 1], F32, tag="dcols", name=f"dcols{pfx}")
    nc.scalar.activation(dcols, f2cols, AF.Exp, scale=ALPHA)
    negd = hw.tile([128, NJC, 1], F32, tag="negd", name=f"negd{pfx}")
    nc.gpsimd.tensor_scalar(negd, dcols, scalar1=-1.0, scalar2=None,
                            op0=OP.mult)
    return vcols, dcols, negd


def _make_wpv_d(nc, hw, wpv, dcols, ncols, pfx):
    """D-scaled wpv rows for the ACT-relu chunks (q in Q_ACT)."""
    wd = hw.tile([128, (NJC // GRP) * len(Q_ACT), ncols + 1], BF16,
                 tag="wpvd", name=f"wpvd{pfx}")
    idx = 0
    for cg in range(NJC // GRP):
        for q in Q_ACT:
            c = cg * GRP + q
            nc.vector.tensor_scalar_mul(wd[:, idx, :], wpv[:, c, :],
                                        dcols[:, c, :])
            idx += 1
    return wd


def _build_program():
    nc = bass.Bass()

    xp = nc.dram_tensor("xp", [2 * NIN, SEQ, RH], BF16, kind="ExternalInput")
    adjTb = nc.dram_tensor("adjTb", [N, R], BF16, kind="ExternalInput")
    lwts = nc.dram_tensor("lwts", [128, 128], F32, kind="ExternalInput")
    bds = nc.dram_tensor("bds", [128, 2], F32, kind="ExternalInput")
    wcat = nc.dram_tensor("wcat", [NHEADS, FEAT, NHID + 2], FP8,
                          kind="ExternalInput")
    wocat = nc.dram_tensor("wocat", [NHEADS * NHID, NCLASS + 2], BF16,
                           kind="ExternalInput")
    outb = nc.dram_tensor("outb", [R, NCLASS], F32, kind="ExternalOutput")

    with tile.TileContext(nc) as tc:
        with tc.tile_pool(name="cst", bufs=1) as cst, \
             tc.tile_pool(name="dram", bufs=1, space="DRAM") as dram:

            ident = cst.tile([128, 128], BF16)
            make_identity(nc, ident)
            ones1 = cst.tile([1, 128], BF16)
            nc.vector.memset(ones1, 1.0)
            maskT = cst.tile([128, NJC, R], BF16)
            hT_own = cst.tile([FEAT, R], FP8)
            hT_full = cst.tile([FEAT, N], FP8)

            g1in = dram.tile([FEAT, R], FP8)
            g1out = dram.tile([NCORES * FEAT, R], FP8, addr_space="Shared")
            g2in = dram.tile([R, NCLASS + 2], BF16)
            g2out = dram.tile([N, NCLASS + 2], BF16, addr_space="Shared")

            # ======== Phase 1: LSTM (own nodes, 2 groups packed) ===========
            with tc.tile_pool(name="p1", bufs=1) as p1, \
                 tc.tile_pool(name="psg", bufs=4, space="PSUM") as psg, \
                 tc.tile_pool(name="hpool0", bufs=SEQ) as hpool0, \
                 tc.tile_pool(name="hpool1", bufs=3) as hpool1, \
                 tc.tile_pool(name="lwork", bufs=6) as lwork:

                xp_sb = p1.tile([2 * NIN, SEQ, RH], BF16)
                nc.sync.dma_start(out=xp_sb, in_=xp[:])
                lw = p1.tile([128, 128], F32)
                nc.sync.dma_start(out=lw, in_=lwts[:])
                bt = p1.tile([128, 2], F32)
                nc.sync.dma_start(out=bt, in_=bds[:])
                w0 = p1.tile([2 * NIN, 128], BF16)
                w0h = p1.tile([24, 128], BF16)
                w1 = p1.tile([24, 128], BF16)
                w1h = p1.tile([24, 128], BF16)
                nc.vector.tensor_copy(w0, lw[0:2 * NIN, :])
                nc.vector.tensor_copy(w0h, lw[32:56, :])
                nc.vector.tensor_copy(w1, lw[64:88, :])
                nc.vector.tensor_copy(w1h, lw[96:120, :])
                b0 = bt[:, 0:1]
                b1 = bt[:, 1:2]

                st0 = _LstmState(0, p1, w0, w0h, b0,
                                 lambda t, hf: xp_sb[:, t, RQ * hf:RQ * (hf + 1)])
                h0s = [[], []]
                st1 = _LstmState(1, p1, w1, w1h, b1,
                                 lambda t, hf: h0s[hf][t])

                # software-pipeline: layer 1 runs one step behind layer 0;
                # the two free-halves are independent chains, so four chains
                # interleave on every engine queue
                adjTr = adjTb[:].rearrange("(c p) r -> p c r", p=128)
                for slot in range(SEQ + 1):
                    for hf in (0, 1):
                        if slot < SEQ:
                            h0s[hf].append(_lstm_step(nc, st0, slot, hf, psg,
                                                      lwork, hpool0))
                        if slot >= 1:
                            t = slot - 1
                            h1 = _lstm_step(nc, st1, t, hf, psg, lwork,
                                            hpool1)
                            h8 = lwork.tile([24, RQ], FP8, tag=f"h8{hf}",
                                            name=f"h8_{t}_{hf}")
                            nc.gpsimd.tensor_copy(h8, h1)
                            co = RQ * hf
                            nc.sync.dma_start(
                                out=g1in[LH * t:LH * (t + 1), co:co + RQ],
                                in_=h8[0:LH, :])
                            nc.sync.dma_start(
                                out=g1in[LH * t:LH * (t + 1),
                                         RH + co:RH + co + RQ],
                                in_=h8[LH:24, :])
                            if t == SEQ - 1 and hf == 1:
                                nc.gpsimd.collective_compute(
                                    "AllGather", OP.bypass,
                                    replica_groups=[list(range(NCORES))],
                                    ins=[g1in[:].opt()],
                                    outs=[g1out[:].opt()])

                nc.sync.dma_start(out=hT_own, in_=g1in[:])
                # mask transfers ride the SP queue inside the g1-gather
                # window; the token copies gate them behind the hT_own
                # readback so they cannot jump ahead of the LSTM's h traffic
                # on the shared DMA engines
                for mg in range(4):
                    nc.vector.tensor_copy(maskT[0:1, 8 * mg:8 * mg + 1, 0:1],
                                          hT_own[0:1, 0:1])
                    nc.sync.dma_start(out=maskT[:, 8 * mg:8 * (mg + 1), :],
                                      in_=adjTr[:, 8 * mg:8 * (mg + 1), :])
                for bb in range(NCORES):
                    nc.sync.dma_start(out=hT_full[:, R * bb:R * (bb + 1)],
                                      in_=g1out[FEAT * bb:FEAT * (bb + 1), :])

            # ======== Phase 2: 8 GAT heads + output GAT layer ===============
            with tc.tile_pool(name="att", bufs=1) as att, \
                 tc.tile_pool(name="pstr", bufs=1, space="PSUM") as pstr, \
                 tc.tile_pool(name="pswh", bufs=2, space="PSUM") as pswh, \
                 tc.tile_pool(name="psf1", bufs=2, space="PSUM") as psf1, \
                 tc.tile_pool(name="pspv", bufs=2, space="PSUM") as pspv, \
                 tc.tile_pool(name="hw", bufs=4) as hw, \
                 tc.tile_pool(name="awork", bufs=3) as awork:

                hcat = att.tile([128, NSUB, NHEADS * NHID], BF16)

                # u for all 8 heads + wc tiles (overlaps the g1 AllGather
                # latency: needs only hT_own)
                wcs = []
                ubs = []
                for h in range(NHEADS):
                    wc = att.tile([FEAT, NHID + 2], FP8, name=f"wc{h}")
                    nc.sync.dma_start(out=wc, in_=wcat[h])
                    wcs.append(wc)
                    ubs.append(_ubcast(nc, psf1, awork, att, ones1,
                                       wc[:, NHID:NHID + 1], hT_own, 1,
                                       1.0 - ALPHA, f"h{h}"))

                def _head_prep(h):
                    wc = wcs[h]
                    whpv = hw.tile([128, NJC, NHID + 1], BF16, tag="whpv",
                                   name=f"whpv{h}")
                    nc.vector.memset(whpv[:, :, NHID:NHID + 1], 1.0)
                    f2cols = hw.tile([128, NJC, 1], F32, tag="f2cols",
                                     name=f"f2cols{h}")
                    # Wh (+f2) for all nodes, 4 chunks per PSUM tile
                    for gq in range(NJC // 4):
                        pw4 = pswh.tile([128, 4, NHID + 2], F32, tag="wh",
                                        name=f"pw{h}_{gq}")
                        for k in range(4):
                            c = 4 * gq + k
                            nc.tensor.matmul(
                                pw4[:, k, :],
                                hT_full[:, 128 * c:128 * (c + 1)],
                                wc, start=True, stop=True)
                        nc.scalar.copy(whpv[:, 4 * gq:4 * (gq + 1), 0:NHID],
                                       pw4[:, :, 0:NHID])
                        nc.scalar.copy(f2cols[:, 4 * gq:4 * (gq + 1), :],
                                       pw4[:, :, NHID + 1:NHID + 2])
                    vcols, dcols, negd = _head_factors(nc, hw, f2cols, f"h{h}")
                    wpvd = _make_wpv_d(nc, hw, whpv, dcols, NHID, f"h{h}")
                    return whpv, vcols, dcols, negd, wpvd

                def _head_post(h, pv):
                    zall = awork.tile([128, NSUB, NHID], F32, tag="zall",
                                      name=f"zall{h}")
                    for s in range(NSUB):
                        rcp = awork.tile([128, 1], F32, tag="rcp",
                                         name=f"rcp{h}_{s}")
                        nc.vector.reciprocal(rcp, pv[:, s, NHID:NHID + 1])
                        nc.vector.tensor_scalar_mul(zall[:, s, :],
                                                    pv[:, s, 0:NHID], rcp)
                    _elu_into(nc, awork, hcat[:, :, NHID * h:NHID * (h + 1)],
                              zall, f"h{h}")

                # heads run in interleaved pairs (group-granular) with prep
                # emitted one pair ahead, keeping every engine queue deep
                preps = {0: _head_prep(0), 1: _head_prep(1)}
                for hp in range(NHEADS // 2):
                    ha, hb = 2 * hp, 2 * hp + 1
                    pa = preps.pop(ha)
                    pb = preps.pop(hb)
                    pva = pspv.tile([128, NSUB, NHID + 1], F32, tag="pv",
                                    name=f"pv_h{ha}")
                    pvb = pspv.tile([128, NSUB, NHID + 1], F32, tag="pv",
                                    name=f"pv_h{hb}")
                    for cg in range(NJC // GRP):
                        _attention_group(nc, awork, pva, cg, ubs[ha], pa[1],
                                         pa[2], pa[3], pa[0], pa[4], maskT,
                                         f"h{ha}")
                        _attention_group(nc, awork, pvb, cg, ubs[hb], pb[1],
                                         pb[2], pb[3], pb[0], pb[4], maskT,
                                         f"h{hb}")
                        if cg == 0 and ha + 2 < NHEADS:
                            preps[ha + 2] = _head_prep(ha + 2)
                        if cg == 1 and hb + 2 < NHEADS:
                            preps[hb + 2] = _head_prep(hb + 2)
                    _head_post(ha, pva)
                    _head_post(hb, pvb)

                # ---- output layer ----
                hcatT = att.tile([128, NSUB, R], BF16)
                for s in range(NSUB):
                    for fc in range(NSUB):
                        ptr = pstr.tile([128, 128], BF16, tag="tr",
                                        name=f"trh{s}_{fc}")
                        nc.tensor.transpose(
                            ptr, hcat[:, s, 128 * fc:128 * (fc + 1)], ident)
                        nc.scalar.copy(hcatT[:, fc, 128 * s:128 * (s + 1)], ptr)

                woc = att.tile([128, NSUB, NCLASS + 2], BF16)
                nc.sync.dma_start(
                    out=woc, in_=wocat.rearrange("(c p) f -> p c f", p=128))

                g2stage = awork.tile([128, NSUB, NCLASS + 2], BF16,
                                     tag="g2stage")
                for s in range(NSUB):
                    pwo = pswh.tile([128, NCLASS + 2], F32, tag="wh",
                                    name=f"pwo{s}")
                    for fc in range(NSUB):
                        nc.tensor.matmul(pwo, hcatT[:, fc, 128 * s:128 * (s + 1)],
                                         woc[:, fc, :], start=(fc == 0),
                                         stop=(fc == NSUB - 1))
                    nc.scalar.copy(g2stage[:, s, :], pwo)
                nc.sync.dma_start(
                    out=g2in[:].rearrange("(c p) f -> p c f", p=128),
                    in_=g2stage)

                ub_o = _ubcast(nc, psf1, awork, att, ones1,
                               woc[:, :, NCLASS:NCLASS + 1], hcatT, NSUB,
                               1.0 - ALPHA, "o")

                nc.gpsimd.collective_compute(
                    "AllGather", OP.bypass,
                    replica_groups=[list(range(NCORES))],
                    ins=[g2in[:].opt()], outs=[g2out[:].opt()])

                wopv = hw.tile([128, NJC, NCLASS + 1], BF16, tag="whpv",
                               name="wopv")
                nc.vector.memset(wopv[:, :, NCLASS:NCLASS + 1], 1.0)
                g2r = g2out[:].rearrange("(c p) f -> p c f", p=128)
                nc.sync.dma_start(out=wopv[:, :, 0:NCLASS],
                                  in_=g2r[:, :, 0:NCLASS])
                f2ob = hw.tile([128, NJC, 1], BF16, tag="f2cols",
                               name="f2ob")
                nc.sync.dma_start(out=f2ob,
                                  in_=g2r[:, :, NCLASS + 1:NCLASS + 2])
                vocols, docols, negdo = _head_factors(nc, hw, f2ob, "o")
                wpvdo = _make_wpv_d(nc, hw, wopv, docols, NCLASS, "o")

                pvo = _attention(nc, awork, pspv, ub_o, vocols, docols,
                                 negdo, wopv, wpvdo, maskT, NCLASS, "o")
                zoall = awork.tile([128, NSUB, NCLASS], F32, tag="zoall")
                for s in range(NSUB):
                    rcp = awork.tile([128, 1], F32, tag="rcp", name=f"rcpo{s}")
                    nc.vector.reciprocal(rcp, pvo[:, s, NCLASS:NCLASS + 1])
                    nc.vector.tensor_scalar_mul(zoall[:, s, :],
                                                pvo[:, s, 0:NCLASS], rcp)
                ziall = awork.tile([128, NSUB, NCLASS], F32, tag="ziall")
                _elu_into(nc, awork, ziall, zoall, "oall")
                for s in range(NSUB):
                    zi = ziall[:, s, :]
                    edump = awork.tile([128, NCLASS], F32, tag="edump",
                                       name=f"ed{s}")
                    ssum = awork.tile([128, 1], F32, tag="ssum", name=f"ss{s}")
                    nc.scalar.activation(edump, zi, AF.Exp, accum_out=ssum)
                    lns = awork.tile([128, 1], F32, tag="lns", name=f"ln{s}")
                    nc.scalar.activation(lns, ssum, AF.Ln)
                    ls = awork.tile([128, NCLASS], F32, tag="ls", name=f"ls{s}")
                    nc.vector.tensor_scalar(ls, zi, scalar1=lns, scalar2=None,
                                            op0=OP.subtract)
                    nc.sync.dma_start(out=outb[128 * s:128 * (s + 1), :],
                                      in_=ls)

    _split_sync_waits(nc)
    return nc


_NC_CACHE = None

_GATE_BASE = {0: 0, 1: 32, 2: 96, 3: 64}  # pytorch i,f,g,o -> partition base


def _pack_wih(w):
    """[4H, in] -> block-diag packed [2*in, 128] bf16: group0 inputs at rows
    0:in -> gate cols base+0:12; group1 at rows in:2*in -> base+12:24."""
    w = np.asarray(w, np.float32)
    nin = w.shape[1]
    out = np.zeros((2 * nin, 128), np.float32)
    for k in range(4):
        base = _GATE_BASE[k]
        blk = w[LH * k:LH * (k + 1), :].T  # [in, 12]
        out[0:nin, base:base + LH] = blk
        out[nin:2 * nin, base + LH:base + 2 * LH] = blk
    return out.astype(ml_dtypes.bfloat16)


def _pack_bias(ba, bb):
    b = np.asarray(ba, np.float32) + np.asarray(bb, np.float32)
    out = np.zeros((128, 1), np.float32)
    for k in range(4):
        base = _GATE_BASE[k]
        out[base:base + LH, 0] = b[LH * k:LH * (k + 1)]
        out[base + LH:base + 2 * LH, 0] = b[LH * k:LH * (k + 1)]
    return out


def kernel(x, adj, Wih0, Whh0, bih0, bhh0, Wih1, Whh1, bih1, bhh1,
           W_heads, a_heads, W_out, a_out):
    global _NC_CACHE
    if _NC_CACHE is None:
        _NC_CACHE = _build_program()
    nc = _NC_CACHE

    x = np.asarray(x, np.float32)
    adj = np.asarray(adj, np.int32)
    W_heads = np.asarray(W_heads, np.float32)
    a_heads = np.asarray(a_heads, np.float32)
    W_out = np.asarray(W_out, np.float32)
    a_out = np.asarray(a_out, np.float32)

    wcat = np.concatenate(
        [W_heads,
         W_heads @ a_heads[:, :NHID, :],
         W_heads @ a_heads[:, NHID:, :]],
        axis=2).astype(ml_dtypes.float8_e4m3fn)
    wocat = np.concatenate(
        [W_out, W_out @ a_out[:NCLASS], W_out @ a_out[NCLASS:]],
        axis=1).astype(ml_dtypes.bfloat16)

    lwts = np.zeros((128, 128), np.float32)
    lwts[0:2 * NIN] = _pack_wih(Wih0).astype(np.float32)
    lwts[32:56] = _pack_wih(Whh0).astype(np.float32)
    lwts[64:88] = _pack_wih(Wih1).astype(np.float32)
    lwts[96:120] = _pack_wih(Whh1).astype(np.float32)
    bds = np.concatenate([_pack_bias(bih0, bhh0),
                          _pack_bias(bih1, bhh1)], axis=1)
    common = {
        "lwts": lwts,
        "bds": np.ascontiguousarray(bds.astype(np.float32)),
        "wcat": np.ascontiguousarray(wcat),
        "wocat": np.ascontiguousarray(wocat),
    }
    adjT = adj.T.astype(ml_dtypes.bfloat16)  # [N(cols j), N(rows i)]
    in_maps = []
    for i in range(NCORES):
        blk = slice(R * i, R * (i + 1))
        xb = x[blk]  # [512, 8, 2]
        xpk = np.concatenate(
            [xb[0:RH].transpose(2, 1, 0), xb[RH:R].transpose(2, 1, 0)],
            axis=0)  # [4, 8, 256]
        in_maps.append({
            "xp": np.ascontiguousarray(xpk).astype(ml_dtypes.bfloat16),
            "adjTb": np.ascontiguousarray(adjT[:, blk]),
            **common,
        })

    res = run_bass_kernel_spmd(nc, in_maps, list(range(NCORES)), **_RUN_KWARGS)
    global _LAST_RESULTS
    _LAST_RESULTS = res
    return np.concatenate([res.results[i]["outb"] for i in range(NCORES)],
                          axis=0)


_RUN_KWARGS = {}
_LAST_RESULTS = None
